# revision 4
# baseline (speedup 1.0000x reference)
"""Trainium2 Bass kernel for nn_AttentionLayer_86629490360750.

reference:
    scores = einsum('bqd,bkd->bqk', query, value)   # no 1/sqrt(d) scaling
    dist   = softmax(scores, axis=-1)
    out    = einsum('bqk,bkd->bqd', dist, value)

Shapes: query/value [4, 4096, 64] fp32.

Sharding: 8 cores; core c handles batch b = c//2, query rows
[h*2048, (h+1)*2048) with h = c%2.  Each core sees its full value[b]
(1 MiB), so there are no collectives.

Per-core algorithm (flash-style, no max subtraction -- scores are
N(0, 64) so |s| < ~55 and exp() stays in fp32 range):
  - transpose Q and V tiles on the PE (fp32 has no DMA transpose) to get
    Q^T [64, 2048] and V^T [64, 4096] in SBUF
  - for each kv tile of 128 rows: S^T tile = V^T_tile.T @ Q^T (PE,
    float32r), exp on ScalarE (PSUM -> SBUF), then accumulate
    ctx^T[65, q] += [V | 1].T @ expS^T (PE, PSUM accumulate).  Row 64 of
    the accumulator is the softmax denominator.
  - tail: transpose ctx^T back to [q, 65] (PE), reciprocal of the
    denominator column (DVE), scale, DMA out.
"""

import os
import sys

import numpy as np

_TRN_REPO = "/opt/trn_rl_repo"
if _TRN_REPO not in sys.path and os.path.isdir(_TRN_REPO):
    sys.path.insert(0, _TRN_REPO)

B, SQ, SKV, D = 4, 4096, 4096, 64
NCORES = 8
CORES_PER_B = NCORES // B          # 2
RQ = SQ // CORES_PER_B             # 2048 query rows per core
P = 128
NKT = SKV // P                     # 32 kv tiles
NQT = RQ // P                      # 16 q tiles
QCH = 1024                         # outer q chunk (psum budget)
NOC = RQ // QCH                    # 2
M2 = D + 1                         # 65: V plus a ones column (denominator)

_CACHE = {}


def _build():
    if "nc" in _CACHE:
        return _CACHE["nc"]

    import concourse.bass as bass  # noqa: F401
    import concourse.mybir as mybir
    import concourse.tile as tile
    from concourse import bacc
    from concourse.masks import make_identity

    f32 = mybir.dt.float32
    f32r = mybir.dt.float32r
    EXP = mybir.ActivationFunctionType.Exp

    nc = bacc.Bacc(
        trn_type="TRN2",
        target_bir_lowering=False,
        debug=False,
        enable_asserts=False,
    )
    q_d = nc.dram_tensor("q", [RQ, D], f32, kind="ExternalInput").ap()
    v_d = nc.dram_tensor("v", [SKV, D], f32, kind="ExternalInput").ap()
    o_d = nc.dram_tensor("o", [RQ, D], f32, kind="ExternalOutput").ap()

    with tile.TileContext(nc) as tc:
        with (
            tc.tile_pool(name="const", bufs=1) as const,
            tc.tile_pool(name="sb", bufs=1) as sb,
            tc.tile_pool(name="es", bufs=3) as es_pool,
            tc.tile_pool(name="outp", bufs=2) as out_pool,
            tc.tile_pool(name="tp", bufs=2, space="PSUM") as tp_pool,
            tc.tile_pool(name="acc", bufs=1, space="PSUM") as acc_pool,
            tc.tile_pool(name="st", bufs=2, space="PSUM") as st_pool,
        ):
            ident = const.tile([P, P], f32)
            make_identity(nc, ident[:])

            q_sb = sb.tile([P, NQT, D], f32)    # natural q tiles
            v_sb = sb.tile([P, NKT, M2], f32r)  # v tiles + ones col
            qt = sb.tile([D, RQ], f32r)         # Q^T
            vt = sb.tile([D, SKV], f32r)        # V^T
            acc_sb = sb.tile([M2, QCH], f32)

            nc.sync.dma_start(q_sb[:], q_d.rearrange("(t p) d -> p t d", p=P))
            nc.sync.dma_start(
                v_sb[:, :, 0:D], v_d.rearrange("(t p) d -> p t d", p=P).bitcast(f32r)
            )
            ones_col = const.tile([P, NKT], f32)
            nc.vector.memset(ones_col[:], 1.0)
            nc.vector.tensor_copy(
                v_sb[:, :, D : D + 1].rearrange("p t o -> p (t o)"), ones_col[:]
            )

            # Q^T / V^T via PE transposes (fp32 cannot DMA-transpose).
            for t in range(NQT):
                tp = tp_pool.tile([P, P], f32)
                nc.tensor.transpose(tp[0:D, :], q_sb[:, t, :], ident[:])
                nc.any.tensor_copy(qt[:, t * P : (t + 1) * P], tp[0:D, :])
            for t in range(NKT):
                tp = tp_pool.tile([P, P], f32)
                nc.tensor.transpose(
                    tp[0:D, :], v_sb[:, t, 0:D].bitcast(f32), ident[:]
                )
                nc.any.tensor_copy(vt[:, t * P : (t + 1) * P], tp[0:D, :])

            for oc in range(NOC):
                acc = acc_pool.tile([M2, QCH], f32)
                for i in range(NKT):
                    st = st_pool.tile([P, QCH], f32)
                    for j in range(QCH // 512):
                        nc.tensor.matmul(
                            st[:, j * 512 : (j + 1) * 512],
                            vt[:, i * P : (i + 1) * P],
                            qt[:, oc * QCH + j * 512 : oc * QCH + (j + 1) * 512],
                            start=True,
                            stop=True,
                        )
                    es = es_pool.tile([P, QCH], f32r)
                    nc.scalar.activation(es[:], st[:], EXP)
                    for j in range(QCH // 512):
                        nc.tensor.matmul(
                            acc[:, j * 512 : (j + 1) * 512],
                            v_sb[:, i, :],
                            es[:, j * 512 : (j + 1) * 512],
                            start=(i == 0),
                            stop=(i == NKT - 1),
                        )

                # tail: normalize + transpose back to [q, d]
                nc.any.tensor_copy(acc_sb[:], acc[:])
                for jt in range(QCH // P):
                    tp = tp_pool.tile([P, P], f32)
                    nc.tensor.transpose(
                        tp[:, 0:M2],
                        acc_sb[:, jt * P : (jt + 1) * P],
                        ident[0:M2, 0:M2],
                    )
                    r = out_pool.tile([P, 1], f32)
                    nc.vector.reciprocal(r[:], tp[:, D : D + 1])
                    ot = out_pool.tile([P, D], f32)
                    nc.vector.tensor_scalar_mul(ot[:], tp[:, 0:D], r[:])
                    row0 = oc * QCH + jt * P
                    nc.sync.dma_start(o_d[row0 : row0 + P, :], ot[:])

    nc.compile()
    _CACHE["nc"] = nc
    return nc


def _in_maps(query, value):
    query = np.ascontiguousarray(np.asarray(query, dtype=np.float32))
    value = np.ascontiguousarray(np.asarray(value, dtype=np.float32))
    maps = []
    for c in range(NCORES):
        b, h = c // CORES_PER_B, c % CORES_PER_B
        maps.append(
            {
                "q": np.ascontiguousarray(query[b, h * RQ : (h + 1) * RQ, :]),
                "v": np.ascontiguousarray(value[b]),
            }
        )
    return maps


def run(query, value, trace=False):
    """Returns (output [4, 4096, 64] fp32, BassKernelResults)."""
    nc = _build()
    from concourse.bass_utils import run_bass_kernel_spmd

    res = run_bass_kernel_spmd(
        nc, _in_maps(query, value), core_ids=list(range(NCORES)), trace=trace
    )
    out = np.empty((B, SQ, D), np.float32)
    for c in range(NCORES):
        b, h = c // CORES_PER_B, c % CORES_PER_B
        out[b, h * RQ : (h + 1) * RQ, :] = res.results[c]["o"]
    return out, res


def kernel(query, value):
    out, _ = run(query, value)
    return out
